# revision 1
# baseline (speedup 1.0000x reference)
"""Trainium2 Bass kernel for DigitsCapsule dynamic routing — I-sharded.

Strategy (8 NeuronCores, model-parallel over input capsules I=1152 ->
144 per core, full batch B=512 on every core):
  Per routing iteration:
    s_part = x_loc @ (e_loc ⊙ w_loc)      (PE, full 128-deep contraction)
    s, S   = AllReduce([s_part; colsum(e_loc)])   (one ~115KB fp16 collective)
    v      = squash(s / S[l])             (full-B squash, 4 chunks)
    T2     = x_locᵀ @ v                   (PE, full-B contraction)
    u_loc  = Σ_{k,O} w_loc ⊙ T2           (DVE on 144 rows only)
    b_loc += u_loc                        (LOCAL — no collective)
  Agreement work (T2/prod/reduce/wc/exp) is 8x smaller per core than the
  batch-parallel layout; the b_ij all-reduce is eliminated entirely.
  Final iteration uses ReduceScatter so each core receives only its own
  64-row output slice.

The 144 local capsules split as 128 ("main") + 16 ("tail"). The tail is
packed (k, i16) into full 128-partition chunks; a constant 0/1 selection
matrix folds the partition-dim k-reduction back to [16, L] on the PE.
"""

import numpy as np

B, I, K, L, O = 512, 1152, 8, 16, 7
NC = 8
IL = I // NC          # 144 capsules per core
BL = B // NC          # 64 output rows per core
F = L * O             # 112
NB = B // 128         # 4 batch chunks
NS = 9                # contraction slots: 8 main (k, i128) + 1 tail (k, i16)
ITERS = 3
RSROW = BL + 1        # rows per ReduceScatter block (64 s rows + S row)

_CACHE = {}


def _build(dt_key, repeat=1, abl=()):
    import concourse.bacc as bacc
    import concourse.mybir as mybir
    import concourse.tile as tile

    DT = {"f32": mybir.dt.float32, "f16": mybir.dt.float16}[dt_key]
    F32 = mybir.dt.float32
    AF = mybir.ActivationFunctionType
    ALU = mybir.AluOpType
    AX = mybir.AxisListType

    nc = bacc.Bacc("TRN2", target_bir_lowering=False, debug=False, num_devices=NC)

    xT_d = nc.dram_tensor("xT", [NS * 128, B], DT, kind="ExternalInput")
    xG_d = nc.dram_tensor("xG", [B, NS * 128], DT, kind="ExternalInput")
    w0_d = nc.dram_tensor("w0", [128, K * F], DT, kind="ExternalInput")
    wt_d = nc.dram_tensor("wt", [128, F], DT, kind="ExternalInput")
    P_d = nc.dram_tensor("P", [128, 16], DT, kind="ExternalInput")
    Pt_d = nc.dram_tensor("Pt", [16, 128], DT, kind="ExternalInput")
    y_d = nc.dram_tensor("y", [16, NB * F], F32, kind="ExternalOutput")

    with tile.TileContext(nc) as tc:
        with (
            tc.tile_pool(name="const", bufs=1) as cpool,
            tc.tile_pool(name="work", bufs=2) as wpool,
            tc.tile_pool(name="small", bufs=2) as spool,
            tc.tile_pool(name="ps_s", bufs=2, space="PSUM") as ps_s,
            tc.tile_pool(name="ps_t2", bufs=1, space="PSUM") as ps_t2,
            tc.tile_pool(name="ps_sm", bufs=3, space="PSUM") as ps_sm,
            tc.tile_pool(name="dram", bufs=2, space="DRAM") as dpool,
        ):
            # ---- one-time input loads, spread across DGE queues ----
            xT = cpool.tile([128, NS * B], DT, tag="xT")
            xt_src = xT_d[:].rearrange("(t p) b -> p t b", p=128)
            xt_dst = xT[:].rearrange("p (t b) -> p t b", t=NS)
            for h in range(3):
                lo, hi = h * 3, (h + 1) * 3
                nc.sync.dma_start(xt_dst[:, lo:hi], xt_src[:, lo:hi])

            xG = cpool.tile([128, NB * NS * 128], DT, tag="xG")
            xg_src = xG_d[:].rearrange("(c p) r -> p c r", p=128)
            xg_dst = xG[:].rearrange("p (c r) -> p c r", c=NB)
            for h in range(2):
                lo, hi = h * 2, (h + 1) * 2
                nc.gpsimd.dma_start(xg_dst[:, lo:hi], xg_src[:, lo:hi])

            w0 = cpool.tile([128, K * F], DT, tag="w0")
            nc.scalar.dma_start(w0[:], w0_d[:])
            wt = cpool.tile([128, F], DT, tag="wt")
            nc.scalar.dma_start(wt[:], wt_d[:])
            P = cpool.tile([128, 16], DT, tag="P")
            nc.scalar.dma_start(P[:], P_d[:])
            Pt = cpool.tile([16, 128], DT, tag="Pt")
            nc.scalar.dma_start(Pt[:], Pt_d[:])

            ones = cpool.tile([128, 1], DT, tag="ones")
            nc.vector.memset(ones[:], 1.0)
            ones1 = cpool.tile([1, 128], F32, tag="ones1")
            nc.vector.memset(ones1[:], 1.0)
            ones8th = cpool.tile([128, 1], DT, tag="ones8th")
            nc.vector.memset(ones8th[:], 0.125)
            srow = cpool.tile([1, F], DT, tag="srow")
            nc.vector.memset(srow[:], 0.0)

            b0 = cpool.tile([128, L], F32, tag="b0")
            bt = cpool.tile([16, L], F32, tag="bt")
            nc.vector.memset(b0[:], 0.0)
            nc.vector.memset(bt[:], 0.0)

            # warm the PE clock-gate during the input-DMA phase
            if "no_warm" not in abl:
                warm0 = cpool.tile([128, 64], DT, tag="warm0")
                nc.vector.memset(warm0[:], 0.0)
                wt0 = ps_sm.tile([1, 64], F32, tag="sm")
                for _ in range(14):
                    nc.tensor.matmul(wt0[:], ones[:], warm0[:],
                                     start=True, stop=True)

            for rep in range(repeat):
             for t in range(ITERS):
                # ---- coupling coefficients (unnormalized e = exp(b/B)) ----
                if t == 0 or "no_wc" in abl:
                    wc0s, wcts = w0, wt
                elif True:
                    e9 = wpool.tile([128, F], DT, tag="e9")
                    nc.scalar.activation(
                        e9[:].rearrange("p (l o) -> p l o", o=O),
                        b0[:].unsqueeze(2).to_broadcast((128, L, O)),
                        AF.Exp, scale=1.0 / B)
                    wc0 = wpool.tile([128, K * F], DT, tag="wc0")
                    nc.vector.tensor_tensor(
                        wc0[:].rearrange("p (k f) -> p k f", k=K),
                        w0[:].rearrange("p (k f) -> p k f", k=K),
                        e9[:].unsqueeze(1).to_broadcast((128, K, F)),
                        op=ALU.mult)
                    # tail: replicate bt across the 8 k-groups via a fp16
                    # selection matmul (Pt.T @ bt16), exp from PSUM
                    bt16 = spool.tile([16, L], DT, tag="bt16")
                    nc.vector.tensor_copy(bt16[:], bt[:])
                    btr_ps = ps_sm.tile([128, L], F32, tag="sm")
                    nc.tensor.matmul(btr_ps[:], Pt[:], bt16[:],
                                     start=True, stop=True)
                    etr = wpool.tile([128, F], DT, tag="etr")
                    nc.scalar.activation(
                        etr[:].rearrange("p (l o) -> p l o", o=O),
                        btr_ps[:].unsqueeze(2).to_broadcast((128, L, O)),
                        AF.Exp, scale=1.0 / B)
                    wct = wpool.tile([128, F], DT, tag="wct")
                    nc.vector.tensor_tensor(wct[:], wt[:], etr[:], op=ALU.mult)
                    # softmax normalizer from e9/etr colsums (etr is 8x
                    # k-replicated, so weight its ones by 1/8)
                    sm_ps = ps_sm.tile([1, F], F32, tag="sm")
                    nc.tensor.matmul(sm_ps[:], ones[:], e9[:],
                                     start=True, stop=False)
                    nc.tensor.matmul(sm_ps[:], ones8th[:], etr[:],
                                     start=False, stop=True)
                    nc.vector.tensor_copy(
                        srow[0:1, 0:L],
                        sm_ps[:].rearrange("p (l o) -> p o l", o=O)[:, 0, :])
                    wc0s, wcts = wc0, wct

                # ---- s_part = x_loc @ (e ⊙ w_loc) : [512, F] in 4 chunks ----
                s16 = wpool.tile([128, NB * F], DT, tag="s16")
                if "no_smm" in abl:
                    nc.vector.memset(s16[:], 0.001)
                s_ps = ps_s.tile([128, NB * F], F32, tag="s_ps")
                for bc in range(NB if "no_smm" not in abl else 0):
                    for s in range(NS):
                        rhs = (wc0s[:, (s * F):(s + 1) * F] if s < K
                               else wcts[:])
                        nc.tensor.matmul(
                            s_ps[:, bc * F:(bc + 1) * F],
                            xT[:, s * B + bc * 128: s * B + (bc + 1) * 128],
                            rhs, start=(s == 0), stop=(s == NS - 1))
                for bc in range(NB if "no_smm" not in abl else 0):
                    nc.scalar.activation(s16[:, bc * F:(bc + 1) * F],
                                         s_ps[:, bc * F:(bc + 1) * F], AF.Copy)

                # ---- collective: AllReduce (t<2) / ReduceScatter (t==2) ----
                if t < ITERS - 1:
                    ar_in = dpool.tile([129, NB * F], DT, tag="ar_in")
                    ar_out = dpool.tile([129, NB * F], DT, tag="ar_out")
                    nc.sync.dma_start(ar_in[0:128, :], s16[:])
                    nc.scalar.dma_start(ar_in[128:129, 0:F], srow[:])
                    if "no_ar" in abl:
                        nc.sync.dma_start(ar_out[:], ar_in[:])
                    else:
                        nc.gpsimd.collective_compute(
                            "AllReduce", ALU.add,
                            replica_groups=[list(range(NC))],
                            ins=[ar_in.opt()], outs=[ar_out.opt()])
                else:
                    rs_in = dpool.tile([NC * 17, NB * F], DT, tag="rs_in")
                    rs_out = dpool.tile([17, NB * F], DT, tag="rs_out")
                    for c in range(NC):
                        nc.sync.dma_start(
                            rs_in[c * 17:c * 17 + 16, :],
                            s16[c * 16:(c + 1) * 16, :])
                        nc.scalar.dma_start(
                            rs_in[c * 17 + 16:(c + 1) * 17, 0:F], srow[:])
                    if "no_ar" in abl:
                        nc.sync.dma_start(rs_out[:], rs_in[0:17, :])
                    else:
                        nc.gpsimd.collective_compute(
                            "ReduceScatter", ALU.add,
                            replica_groups=[list(range(NC))],
                            ins=[rs_in.opt()], outs=[rs_out.opt()])

                # keep PE's clock-gate open through the collective window:
                # dummy matmuls reading s16 (pins them to this window)
                if "no_warm" not in abl:
                    wrm = ps_sm.tile([1, NB * F], F32, tag="sm")
                    for _ in range(10):
                        nc.tensor.matmul(wrm[:], ones[:], s16[:],
                                         start=True, stop=True)

                if t == ITERS - 1:
                    # ---- final: own 16-partition slice (j, bc, l, o) ----
                    sv = wpool.tile([16, NB * F], DT, tag="sv")
                    nc.sync.dma_start(sv[:], rs_out[0:16, :])
                    svSf = spool.tile([1, F], DT, tag="svSf")
                    nc.scalar.dma_start(svSf[:], rs_out[16:17, 0:F])
                    invS = spool.tile([1, L], F32, tag="invS")
                    nc.vector.reciprocal(invS[:], svSf[0:1, 0:L])
                    ibc_ps = ps_sm.tile([16, L], F32, tag="sm")
                    nc.tensor.matmul(ibc_ps[:], ones1[:, 0:16], invS[:],
                                     start=True, stop=True)
                    v_out = wpool.tile([16, NB * F], F32, tag="v_out")
                    for bc in range(NB):
                        sb = sv[:, bc * F:(bc + 1) * F]
                        s_n = wpool.tile([16, F], F32, tag="s_n")
                        nc.vector.tensor_tensor(
                            s_n[:].rearrange("p (l o) -> p l o", o=O),
                            sb.rearrange("p (l o) -> p l o", o=O),
                            ibc_ps[:].unsqueeze(2).to_broadcast((16, L, O)),
                            op=ALU.mult)
                        sq2 = wpool.tile([16, F], F32, tag="sq2")
                        nc.vector.tensor_tensor(sq2[:], s_n[:], s_n[:],
                                                op=ALU.mult)
                        sq = spool.tile([16, L], F32, tag="sq")
                        nc.vector.tensor_reduce(
                            sq[:], sq2[:].rearrange("p (l o) -> p l o", o=O),
                            axis=AX.X, op=ALU.add)
                        nrm = spool.tile([16, L], F32, tag="nrm")
                        nc.scalar.activation(nrm[:], sq[:], AF.Sqrt)
                        d1 = spool.tile([16, L], F32, tag="d1")
                        nc.vector.tensor_scalar_add(d1[:], sq[:], 1.0)
                        rin = spool.tile([16, L], F32, tag="rin")
                        nc.vector.reciprocal(rin[:], d1[:])
                        fm = spool.tile([16, L], F32, tag="fm")
                        nc.vector.tensor_tensor(fm[:], nrm[:], rin[:],
                                                op=ALU.mult)
                        nc.vector.tensor_tensor(
                            v_out[:, bc * F:(bc + 1) * F]
                            .rearrange("p (o l) -> p l o", o=O),
                            s_n[:].rearrange("p (l o) -> p l o", o=O),
                            fm[:].unsqueeze(2).to_broadcast((16, L, O)),
                            op=ALU.mult)
                    nc.sync.dma_start(y_d[:], v_out[:])
                    continue

                # ---- v = squash(s/S) over full B, fused with T2 accum ----
                sfull = wpool.tile([128, NB * F], DT, tag="sfull")
                invSb = None
                if "no_unl" in abl:
                    nc.vector.memset(sfull[:], 0.001)
                if t > 0 and "no_unl" not in abl:
                    # S row first so invS is ready before the chunks land
                    svS = spool.tile([1, F], DT, tag="svS")
                    nc.scalar.dma_start(svS[:], ar_out[128:129, 0:F])
                    invS2 = spool.tile([1, L], F32, tag="invS2")
                    nc.vector.reciprocal(invS2[:], svS[0:1, 0:L])
                    ib_ps = ps_sm.tile([128, L], F32, tag="sm")
                    nc.tensor.matmul(ib_ps[:], ones1[:], invS2[:],
                                     start=True, stop=True)
                    invSb = spool.tile([128, L], F32, tag="invSb")
                    nc.vector.tensor_copy(invSb[:], ib_ps[:])
                if "no_unl" not in abl:
                    nc.sync.dma_start(sfull[:], ar_out[0:128, :])

                if "no_u" in abl:
                    continue
                t2h = [ps_t2.tile([128, 4 * F], F32, tag=f"t2h{h}",
                                  name=f"t2h{h}")
                       for h in range(2)]
                t2t = ps_t2.tile([128, F], F32, tag="t2t")
                for bc in range(NB):
                    sb = sfull[:, bc * F:(bc + 1) * F]
                    if "no_sq" in abl:
                        v16 = wpool.tile([128, F], DT, tag="v16")
                        nc.vector.memset(v16[:], 0.001)
                        xgb = xG[:, bc * NS * 128:]
                        for h in range(2):
                            for kk in range(4):
                                k = h * 4 + kk
                                nc.tensor.matmul(
                                    t2h[h][:, kk * F:(kk + 1) * F],
                                    xgb[:, k * 128:(k + 1) * 128], v16[:],
                                    start=(bc == 0), stop=(bc == NB - 1))
                        nc.tensor.matmul(
                            t2t[:], xgb[:, K * 128:NS * 128], v16[:],
                            start=(bc == 0), stop=(bc == NB - 1))
                        continue
                    ssq = spool.tile([128, L], F32, tag="ssq")
                    sq2b = wpool.tile([128, F], F32, tag="sq2b")
                    nc.vector.tensor_tensor(sq2b[:], sb, sb, op=ALU.mult)
                    nc.vector.tensor_reduce(
                        ssq[:], sq2b[:].rearrange("p (l o) -> p l o", o=O),
                        axis=AX.X, op=ALU.add)
                    if t == 0 or invSb is None:
                        sqn = spool.tile([128, L], F32, tag="sqn")
                        nc.vector.tensor_scalar_mul(sqn[:], ssq[:],
                                                    1.0 / (I * I))
                    else:
                        sqn = spool.tile([128, L], F32, tag="sqn")
                        nc.vector.tensor_tensor(sqn[:], ssq[:], invSb[:],
                                                op=ALU.mult)
                        nc.vector.tensor_tensor(sqn[:], sqn[:], invSb[:],
                                                op=ALU.mult)
                    nrmb = spool.tile([128, L], F32, tag="nrmb")
                    nc.scalar.activation(nrmb[:], sqn[:], AF.Sqrt)
                    d1b = spool.tile([128, L], F32, tag="d1b")
                    nc.vector.tensor_scalar_add(d1b[:], sqn[:], 1.0)
                    rb = spool.tile([128, L], F32, tag="rb")
                    nc.vector.reciprocal(rb[:], d1b[:])
                    g = spool.tile([128, L], F32, tag="g")
                    nc.vector.tensor_tensor(g[:], nrmb[:], rb[:],
                                            op=ALU.mult)
                    if t == 0 or invSb is None:
                        nc.vector.tensor_scalar_mul(g[:], g[:], 1.0 / I)
                    else:
                        nc.vector.tensor_tensor(g[:], g[:], invSb[:],
                                                op=ALU.mult)
                    v16 = wpool.tile([128, F], DT, tag="v16")
                    nc.vector.tensor_tensor(
                        v16[:].rearrange("p (l o) -> p l o", o=O),
                        sb.rearrange("p (l o) -> p l o", o=O),
                        g[:].unsqueeze(2).to_broadcast((128, L, O)),
                        op=ALU.mult)
                    # T2 accumulation for this batch chunk
                    xgb = xG[:, bc * NS * 128:]
                    for h in range(2):
                        for kk in range(4):
                            k = h * 4 + kk
                            nc.tensor.matmul(
                                t2h[h][:, kk * F:(kk + 1) * F],
                                xgb[:, k * 128:(k + 1) * 128], v16[:],
                                start=(bc == 0), stop=(bc == NB - 1))
                    nc.tensor.matmul(
                        t2t[:], xgb[:, K * 128:NS * 128], v16[:],
                        start=(bc == 0), stop=(bc == NB - 1))

                # ---- agreement: u = sum_{k,O} w ⊙ T2 (local only) ----
                prod = wpool.tile([128, K * F], DT, tag="prod")
                for h in range(2):
                    nc.vector.tensor_tensor(
                        prod[:, h * 4 * F:(h + 1) * 4 * F], t2h[h][:],
                        w0[:, h * 4 * F:(h + 1) * 4 * F], op=ALU.mult)
                prodt = wpool.tile([128, F], DT, tag="prodt")
                nc.vector.tensor_tensor(prodt[:], t2t[:], wt[:], op=ALU.mult)
                u0 = wpool.tile([128, L], F32, tag="u0")
                with nc.allow_low_precision("fp16 agreement; b re-acc fp32"):
                    nc.vector.tensor_reduce(
                        u0[:],
                        prod[:].rearrange("p (k l o) -> p l k o", k=K, l=L),
                        axis=AX.XY, op=ALU.add)
                    qt = wpool.tile([128, L], DT, tag="qt")
                    nc.vector.tensor_reduce(
                        qt[:], prodt[:].rearrange("p (l o) -> p l o", o=O),
                        axis=AX.X, op=ALU.add)
                ut_ps = ps_sm.tile([16, L], F32, tag="sm")
                nc.tensor.matmul(ut_ps[:], P[:], qt[:], start=True, stop=True)
                if t == 0:
                    nc.vector.tensor_copy(b0[:], u0[:])
                    nc.vector.tensor_copy(bt[:], ut_ps[:])
                else:
                    nc.vector.tensor_add(b0[:], b0[:], u0[:])
                    nc.vector.tensor_add(bt[:], bt[:], ut_ps[:])

    nc.compile()
    return nc


def _get_nc(dt_key, repeat=1, abl=()):
    key = (dt_key, repeat, tuple(sorted(abl)))
    if key not in _CACHE:
        _CACHE[key] = _build(dt_key, repeat, abl)
    return _CACHE[key]


def _prep_inputs(x, w, np_dt):
    """Per-core input maps for the I-sharded layout."""
    in_maps = []
    Pm = np.tile(np.eye(16, dtype=np_dt), (8, 1))          # [128, 16]
    Ptm = np.ascontiguousarray(Pm.T)                       # [16, 128]
    for c in range(NC):
        xl = x[:, c * IL:(c + 1) * IL, :].astype(np_dt)    # [512, 144, 8]
        main = xl[:, :128, :]                              # [512, 128, 8]
        tail = xl[:, 128:, :]                              # [512, 16, 8]
        # slots 0..7: (k, i128); slot 8: (k, i16)
        xT = np.empty((NS * 128, B), np_dt)
        for k in range(K):
            xT[k * 128:(k + 1) * 128, :] = main[:, :, k].T
        xT[K * 128:, :] = tail.transpose(2, 1, 0).reshape(128, B)
        xG = np.ascontiguousarray(xT.T)
        wl = w[c * IL:(c + 1) * IL].astype(np_dt)          # [144, 8, 16, 7]
        w0 = np.ascontiguousarray(wl[:128].reshape(128, K * F))
        wtl = np.ascontiguousarray(
            wl[128:].transpose(1, 0, 2, 3).reshape(128, F))
        in_maps.append({"xT": np.ascontiguousarray(xT), "xG": xG,
                        "w0": w0, "wt": wtl, "P": Pm, "Pt": Ptm})
    return in_maps


def kernel(x, w, _dt="f16", _trace=False):
    x = np.asarray(x, dtype=np.float32)
    w = np.asarray(w, dtype=np.float32)
    np_dt = {"f32": np.float32, "f16": np.float16}[_dt]

    nc = _get_nc(_dt)
    in_maps = _prep_inputs(x, w, np_dt)

    from concourse.bass_utils import run_bass_kernel_spmd
    res = run_bass_kernel_spmd(
        nc, in_maps, core_ids=list(range(NC)), trace=_trace)
    kernel.last_result = res
    out = np.empty((B, O, L), np.float32)
    for c in range(NC):
        yc = res.results[c]["y"].reshape(16, NB, O, L)
        for bc in range(NB):
            out[bc * 128 + 16 * c: bc * 128 + 16 * (c + 1)] = yc[:, bc]
    return out.astype(np.float32)


kernel.last_result = None

